# revision 8
# baseline (speedup 1.0000x reference)
"""Trainium2 Bass kernel for nn_DEAttention_Module (dense channel-attention).

Math (per batch b, with X = x[b] viewed as (C=512, N=4096), row-major):
    q = Wq @ X ; k = Wk @ X ; v = Wv @ X            (1x1 convs, biases zero)
    The torch-style .view(B, N, C) is a raw reinterpret: chunk c of 512
    columns of q becomes rows [512c, 512c+512) of q_resh.  Hence with
    Xk = X[:, 512k:512(k+1)]:
        energy = sum_k (Wq Xk)^T (Wk Xk) = sum_k Xk^T M Xk,  M = Wq^T Wk
        attn   = softmax(energy, axis=-1)
        y[:, 512k:512(k+1)] = gamma * (Wv Xk) attn^T + Xk
    M is folded on the host (512x512 fp64->fp32), which merges the q and k
    projections into one GEMM chain: Hk = M Xk ; energy += Xk^T Hk.

Precision: PE fp32 matmul runs at 1/4 rate; float32r (TF32-like, 11-bit
mantissa, RTN) runs at full rate for free-dim >= 256.  With comp=True each
f32r GEMM on the energy path is error-compensated (operand = rounded hi +
rounded residual, dropping only the lo*lo term), recovering ~fp32 accuracy
at 3x f32r cost (still ~1.3x faster than native fp32).  The V-side GEMMs
run plain f32r; their error is damped linearly by gamma and the softmax
row-sum normalization (measured end-to-end ~7e-5 of output absmax).

Sharding: data-parallel over batch B=8 across the 8 cores (one batch per
core); the small CxC weights are replicated.
"""
import sys
from contextlib import ExitStack

sys.path.insert(0, "/opt/trn_rl_repo")

import numpy as np

import concourse.bacc as bacc
import concourse.bass as bass
import concourse.tile as tile
from concourse import mybir
from concourse.bass_utils import run_bass_kernel_spmd
from concourse.masks import make_identity

f32 = mybir.dt.float32
f32r = mybir.dt.float32r

P = 128   # SBUF partitions
T = 4     # channel tiles (C = T*P = 512)
CH = 8    # column chunks (N = CH*S = 4096)
S = 512   # chunk width = matmul moving free dim
C = 512
N = 4096

COMP = True  # error-compensated energy path (V3); False = plain f32r (V1)


def build(comp=COMP):
    nc = bacc.Bacc("TRN2", target_bir_lowering=False, debug=False)
    x_d = nc.dram_tensor("x", [C, N], f32, kind="ExternalInput")
    mt_d = nc.dram_tensor("mt", [C, C], f32, kind="ExternalInput")     # (Wq^T Wk)^T
    wvt_d = nc.dram_tensor("wvt", [C, C], f32, kind="ExternalInput")   # Wv^T
    gam_d = nc.dram_tensor("gam", [P, 1], f32, kind="ExternalInput")
    bvb_d = nc.dram_tensor("bvb", [P, S], f32, kind="ExternalInput")   # bv bcast
    y_d = nc.dram_tensor("y", [C, N], f32, kind="ExternalOutput")

    Exp = mybir.ActivationFunctionType.Exp
    mult = mybir.AluOpType.mult
    add_ = mybir.AluOpType.add
    maxop = mybir.AluOpType.max
    AX = mybir.AxisListType.X

    with tile.TileContext(nc) as tc:
        with (
            tc.tile_pool(name="consts", bufs=1) as consts,
            tc.tile_pool(name="xtmp", bufs=3) as xtmpp,
            tc.tile_pool(name="xr", bufs=3) as xrp,
            tc.tile_pool(name="hkr", bufs=2) as hkrp,
            tc.tile_pool(name="vkt", bufs=3) as vktp,
            tc.tile_pool(name="yout", bufs=4) as youtp,
            tc.tile_pool(name="pse", bufs=4, space="PSUM") as pse,
            tc.tile_pool(name="pss", bufs=4, space="PSUM") as pss,
            ExitStack() as ctx_pools,
        ):
            ident = consts.tile([P, P], f32)
            make_identity(nc, ident)
            gammab = consts.tile([P, 1], f32)
            nc.sync.dma_start(out=gammab, in_=gam_d[:, :])
            bvb = consts.tile([P, S], f32)
            nc.sync.dma_start(out=bvb, in_=bvb_d[:, :])

            mtr = consts.tile([P, T, S], f32r)
            wvtr = consts.tile([P, T, S], f32r)
            mte = consts.tile([P, T, S], f32r, name="mte", tag="mte") if comp else None
            with tc.tile_pool(name="wstage", bufs=1) as wst:
                mtsb = wst.tile([P, T, S], f32)
                wvsb = wst.tile([P, T, S], f32)
                for t in range(T):
                    nc.sync.dma_start(out=mtsb[:, t, :], in_=mt_d[P * t:P * (t + 1), :])
                    nc.sync.dma_start(out=wvsb[:, t, :], in_=wvt_d[P * t:P * (t + 1), :])
                for t in range(T):
                    nc.vector.tensor_copy(mtr[:, t, :], mtsb[:, t, :])
                    nc.vector.tensor_copy(wvtr[:, t, :], wvsb[:, t, :])
                    if comp:
                        nc.vector.tensor_sub(
                            mte[:, t, :], mtsb[:, t, :], mtr[:, t, :].bitcast(f32)
                        )

            attn = consts.tile([P, T, S], f32)
            attnt = consts.tile([P, T, S], f32r)
            negmax = consts.tile([P, T], f32)
            sums = consts.tile([P, T], f32)
            rsum = consts.tile([P, T], f32)

            xep = hkep = None
            if comp:
                xep = ctx_pools.enter_context(tc.tile_pool(name="xe", bufs=2))
                hkep = ctx_pools.enter_context(tc.tile_pool(name="hke", bufs=2))

            en = [pse.tile([P, S], f32, name=f"en{i}", tag="energy") for i in range(T)]

            # ---------------- phase B: energy = sum_k Xk^T (M Xk) ----------------
            for k in range(CH):
                xt = xtmpp.tile([P, T, S], f32, tag="xt")
                for t in range(T):
                    nc.sync.dma_start(
                        out=xt[:, t, :], in_=x_d[P * t:P * (t + 1), S * k:S * (k + 1)]
                    )
                xr = xrp.tile([P, T, S], f32r, tag="xr")
                for t in range(T):
                    nc.vector.tensor_copy(xr[:, t, :], xt[:, t, :])
                xe = None
                if comp:
                    xe = xep.tile([P, T, S], f32r, tag="xe")
                    for t in range(T):
                        nc.vector.tensor_sub(
                            xe[:, t, :], xt[:, t, :], xr[:, t, :].bitcast(f32)
                        )

                hkr = hkrp.tile([P, T, S], f32r, tag="hkr")
                hke = hkep.tile([P, T, S], f32r, name="hke", tag="hke") if comp else None
                hterms = [(mtr, xr)] + ([(mtr, xe), (mte, xr)] if comp else [])
                for c1 in range(T):
                    hk_ps = pss.tile([P, S], f32, tag="ps")
                    nmm = len(hterms) * T
                    i = 0
                    for stat_t, mov_t in hterms:
                        for c2 in range(T):
                            nc.tensor.matmul(
                                hk_ps,
                                stat_t[:, c2, P * c1:P * (c1 + 1)],
                                mov_t[:, c2, :],
                                start=(i == 0),
                                stop=(i == nmm - 1),
                            )
                            i += 1
                    nc.scalar.copy(hkr[:, c1, :], hk_ps)
                    if comp:
                        nc.vector.tensor_sub(
                            hke[:, c1, :], hk_ps, hkr[:, c1, :].bitcast(f32)
                        )

                eterms = [(xr, hkr)] + ([(xe, hkr), (xr, hke)] if comp else [])
                for si in range(T):
                    nmm = len(eterms) * T
                    i = 0
                    for stat_t, mov_t in eterms:
                        for ct in range(T):
                            nc.tensor.matmul(
                                en[si],
                                stat_t[:, ct, P * si:P * (si + 1)],
                                mov_t[:, ct, :],
                                start=(k == 0 and i == 0),
                                stop=(k == CH - 1 and i == nmm - 1),
                                skip_group_check=True,
                            )
                            i += 1

            # ---------------- softmax over j (free dim) ----------------
            for si in range(T):
                nc.vector.tensor_reduce(
                    out=negmax[:, si:si + 1], in_=en[si], axis=AX, op=maxop, negate=True
                )
                nc.scalar.activation(
                    out=attn[:, si, :],
                    in_=en[si],
                    func=Exp,
                    bias=negmax[:, si:si + 1],
                    scale=1.0,
                    accum_out=sums[:, si:si + 1],
                )
                nc.vector.reciprocal(out=rsum[:, si:si + 1], in_=sums[:, si:si + 1])
                nc.vector.tensor_scalar_mul(
                    attn[:, si, :], attn[:, si, :], rsum[:, si:si + 1]
                )

            # ---------------- phase E: y_k = gamma * (Wv Xk) attn^T + Xk --------
            def e_front(k):
                """x chunk re-load + round, VkT = (Wv Xk)^T via stat=Xk blocks."""
                xt2 = xtmpp.tile([P, T, S], f32, tag="xt")
                for t in range(T):
                    nc.sync.dma_start(
                        out=xt2[:, t, :], in_=x_d[P * t:P * (t + 1), S * k:S * (k + 1)]
                    )
                xr2 = xrp.tile([P, T, S], f32r, tag="xr")
                for t in range(T):
                    nc.vector.tensor_copy(xr2[:, t, :], xt2[:, t, :])
                vkt = vktp.tile([P, T, S], f32r, tag="vkt")
                for ms in range(T):
                    v_ps = pss.tile([P, S], f32, tag="ps")
                    for ct in range(T):
                        nc.tensor.matmul(
                            v_ps,
                            xr2[:, ct, P * ms:P * (ms + 1)],
                            wvtr[:, ct, :],
                            start=(ct == 0),
                            stop=(ct == T - 1),
                        )
                    nc.vector.tensor_tensor(
                        out=vkt[:, ms, :], in0=v_ps, in1=bvb, op=add_
                    )
                return xt2, vkt

            def e_back(k, xt2, vkt):
                for os in range(T):
                    o_ps = pss.tile([P, S], f32, tag="ps")
                    for jt in range(T):
                        nc.tensor.matmul(
                            o_ps,
                            vkt[:, jt, P * os:P * (os + 1)],
                            attnt[:, jt, :],
                            start=(jt == 0),
                            stop=(jt == T - 1),
                        )
                    yo = youtp.tile([P, S], f32, tag="yo")
                    nc.vector.scalar_tensor_tensor(
                        out=yo,
                        in0=o_ps,
                        scalar=gammab[:, 0:1],
                        in1=xt2[:, os, :],
                        op0=mult,
                        op1=add_,
                    )
                    nc.sync.dma_start(
                        out=y_d[P * os:P * (os + 1), S * k:S * (k + 1)], in_=yo
                    )

            # two chunks of V-work first so PE stays busy during softmax
            pending = [e_front(0), e_front(1)]

            # attn^T via PE transposes (after softmax), rounded to f32r on copy-out
            for jt in range(T):
                for si in range(T):
                    trp = pss.tile([P, P], f32, tag="ps")
                    nc.tensor.transpose(trp, attn[:, si, P * jt:P * (jt + 1)], ident)
                    nc.scalar.copy(attnt[:, jt, P * si:P * (si + 1)], trp)

            for k in range(CH):
                if k + 2 < CH:
                    pending.append(e_front(k + 2))
                xt2, vkt = pending.pop(0)
                e_back(k, xt2, vkt)

    nc.compile()
    return nc


_NC_CACHE = {}


def _get_nc(comp=COMP):
    if comp not in _NC_CACHE:
        _NC_CACHE[comp] = build(comp)
    return _NC_CACHE[comp]


def kernel(x, Wq, bq, Wk, bk, Wv, bv, gamma, comp=COMP):
    x = np.ascontiguousarray(np.asarray(x, np.float32))
    B = x.shape[0]
    assert x.shape == (B, C, 64, 64) and B == 8, x.shape
    if np.any(np.asarray(bq)) or np.any(np.asarray(bk)):
        raise NotImplementedError("nonzero q/k biases not supported")

    # host-side weight folding: M^T = Wk^T Wq in fp64 (134 MFLOP, ~0.2% of
    # the module's FLOPs) merges the q/k projections into one GEMM chain.
    mt = (np.asarray(Wk, np.float64).T @ np.asarray(Wq, np.float64)).astype(np.float32)
    wvt = np.ascontiguousarray(np.asarray(Wv, np.float32).T)
    gam = np.full((P, 1), np.float32(np.asarray(gamma).reshape(-1)[0]), np.float32)
    bvb = np.ascontiguousarray(
        np.broadcast_to(np.asarray(bv, np.float32), (P, S))
    )

    nc = _get_nc(comp)
    in_maps = [
        {
            "x": np.ascontiguousarray(x[b].reshape(C, N)),
            "mt": mt,
            "wvt": wvt,
            "gam": gam,
            "bvb": bvb,
        }
        for b in range(B)
    ]
    res = run_bass_kernel_spmd(nc, in_maps, core_ids=list(range(B)))
    out = np.stack([res.results[b]["y"].reshape(C, 64, 64) for b in range(B)])
    return out.astype(np.float32)


# revision 11
# speedup vs baseline: 2.1159x; 2.1159x over previous
"""Trainium2 Bass kernel for nn_DEAttention_Module (dense channel-attention).

Math (per batch b, with X = x[b] viewed as (C=512, N=4096), row-major):
    q = Wq @ X ; k = Wk @ X ; v = Wv @ X            (1x1 convs)
    The torch-style .view(B, N, C) is a raw reinterpret: chunk k of 512
    columns of q becomes rows [512k, 512k+512) of q_resh.  Hence with
    Xk = X[:, 512k:512(k+1)]:
        energy = sum_k (Wq Xk)^T (Wk Xk) = sum_k Xk^T M Xk,  M = Wq^T Wk
        attn   = softmax(energy, axis=-1)
        y[:, 512k:512(k+1)] = gamma * (Wv Xk) attn^T + Xk
    M is folded on the host (512x512, fp64->fp32), which merges the q and k
    projections into one GEMM chain: Hk = M Xk ; energy += Xk^T Hk.

Precision: PE fp32 matmul runs at 1/4 rate; float32r (TF32-like: fp32 with
11-bit mantissa, RTN) runs at full rate for moving free-dim >= 256.  x is
held in SBUF as a rounded hi/lo f32r pair (xr + xe, exact to ~2^-24): both
halves are direct matmul operands and their sum reconstructs x for the
residual add.  With comp=True each f32r GEMM on the energy path is
error-compensated (hi*hi + hi*lo + lo*hi), recovering ~fp32 accuracy at 3x
f32r cost (still 1.33x faster than native fp32).  The V-side GEMMs run
plain f32r; that error is damped by gamma and the softmax row-sum
normalization (measured end-to-end ~8e-5 of output absmax).

Sharding: data-parallel over batch B=8 across the 8 cores (one batch per
core); the small CxC weights are replicated.
"""
import sys
from contextlib import ExitStack

sys.path.insert(0, "/opt/trn_rl_repo")

import numpy as np

import concourse.bacc as bacc
import concourse.bass as bass
import concourse.tile as tile
from concourse import mybir
from concourse.bass_utils import run_bass_kernel_spmd
from concourse.masks import make_identity

f32 = mybir.dt.float32
f32r = mybir.dt.float32r

P = 128   # SBUF partitions
T = 4     # channel tiles (C = T*P = 512)
CH = 8    # column chunks (N = CH*S = 4096)
S = 512   # chunk width = matmul moving free dim
C = 512
N = 4096

COMP = True  # error-compensated energy path (V3); False = plain f32r (V1)


def build(comp=COMP, has_bv=False):
    nc = bacc.Bacc("TRN2", target_bir_lowering=False, debug=False)
    x_d = nc.dram_tensor("x", [C, N], f32, kind="ExternalInput")
    mt_d = nc.dram_tensor("mt", [C, C], f32, kind="ExternalInput")     # (Wq^T Wk)^T
    wvt_d = nc.dram_tensor("wvt", [C, C], f32, kind="ExternalInput")   # Wv^T
    gam_d = nc.dram_tensor("gam", [P, 1], f32, kind="ExternalInput")
    bvb_d = nc.dram_tensor("bvb", [P, S], f32, kind="ExternalInput")   # bv bcast
    y_d = nc.dram_tensor("y", [C, N], f32, kind="ExternalOutput")

    Exp = mybir.ActivationFunctionType.Exp
    mult = mybir.AluOpType.mult
    add_ = mybir.AluOpType.add
    maxop = mybir.AluOpType.max
    AX = mybir.AxisListType.X

    with tile.TileContext(nc) as tc:
        with (
            tc.tile_pool(name="consts", bufs=1) as consts,
            tc.tile_pool(name="xtmp", bufs=2) as xtmpp,
            tc.tile_pool(name="hk", bufs=3) as hkp,
            tc.tile_pool(name="vkt", bufs=3) as vktp,
            tc.tile_pool(name="pse", bufs=4, space="PSUM") as pse,
            tc.tile_pool(name="pss", bufs=4, space="PSUM") as pss,
            ExitStack() as ctx_pools,
        ):
            # --- weights first: the first PE work (Hk of chunk 0) needs mtr ---
            mtr = consts.tile([P, T, S], f32r)
            mte = consts.tile([P, T, S], f32r, name="mte", tag="mte") if comp else None
            wvtr = consts.tile([P, T, S], f32r)
            mtsb = xtmpp.tile([P, T, S], f32, tag="xt", name="mtsb")
            for t in range(T):
                nc.sync.dma_start(out=mtsb[:, t, :], in_=mt_d[P * t:P * (t + 1), :])
            for t in range(T):
                nc.vector.tensor_copy(mtr[:, t, :], mtsb[:, t, :])
                if comp:
                    nc.vector.tensor_sub(
                        mte[:, t, :], mtsb[:, t, :], mtr[:, t, :].bitcast(f32)
                    )
            wvsb = xtmpp.tile([P, T, S], f32, tag="xt", name="wvsb")
            for t in range(T):
                nc.sync.dma_start(out=wvsb[:, t, :], in_=wvt_d[P * t:P * (t + 1), :])
            for t in range(T):
                nc.vector.tensor_copy(wvtr[:, t, :], wvsb[:, t, :])

            ident = consts.tile([P, P], f32)
            make_identity(nc, ident)
            gammab = consts.tile([P, 1], f32)
            nc.sync.dma_start(out=gammab, in_=gam_d[:, :])
            bvb = None
            if has_bv:
                bvb = consts.tile([P, S], f32, name="bvb", tag="bvb")
                nc.sync.dma_start(out=bvb, in_=bvb_d[:, :])

            # rounded x resident (f32r hi part); lo residual is per-chunk
            xr = consts.tile([P, T, N], f32r)

            attn = consts.tile([P, T, S], f32)
            attnt = consts.tile([P, T, S], f32r)
            negmax = consts.tile([P, T], f32)
            sums = consts.tile([P, T], f32)
            rsum = consts.tile([P, T], f32)

            xep = None
            if comp:
                xep = ctx_pools.enter_context(tc.tile_pool(name="xe", bufs=2))

            en = [pse.tile([P, S], f32, name=f"en{i}", tag="energy") for i in range(T)]

            # ---------------- phase B: energy = sum_k Xk^T (M Xk) ----------------
            for k in range(CH):
                sl = slice(S * k, S * (k + 1))
                xt = xtmpp.tile([P, T, S], f32, tag="xt", name="xt")
                for t in range(T):
                    nc.sync.dma_start(
                        out=xt[:, t, :], in_=x_d[P * t:P * (t + 1), sl]
                    )
                for t in range(T):
                    nc.vector.tensor_copy(xr[:, t, sl], xt[:, t, :])
                xe = None
                if comp:
                    xe = xep.tile([P, T, S], f32r, name="xe", tag="xe")
                    for t in range(T):
                        nc.vector.tensor_sub(
                            xe[:, t, :], xt[:, t, :], xr[:, t, sl].bitcast(f32)
                        )

                hkr = hkp.tile([P, T, S], f32r, tag="hk", name="hkr")
                hke = (
                    hkp.tile([P, T, S], f32r, name="hke", tag="hk") if comp else None
                )
                hterms = [(mtr, xr)] + ([(mtr, xe), (mte, xr)] if comp else [])
                for c1 in range(T):
                    hk_ps = pss.tile([P, S], f32, tag="ps", name="hk_ps")
                    nmm = len(hterms) * T
                    i = 0
                    for stat_t, mov_t in hterms:
                        for c2 in range(T):
                            mv = mov_t[:, c2, sl] if mov_t is xr else mov_t[:, c2, :]
                            nc.tensor.matmul(
                                hk_ps,
                                stat_t[:, c2, P * c1:P * (c1 + 1)],
                                mv,
                                start=(i == 0),
                                stop=(i == nmm - 1),
                            )
                            i += 1
                    nc.scalar.copy(hkr[:, c1, :], hk_ps)
                    if comp:
                        nc.vector.tensor_sub(
                            hke[:, c1, :], hk_ps, hkr[:, c1, :].bitcast(f32)
                        )

                eterms = [(xr, hkr)] + ([(xe, hkr), (xr, hke)] if comp else [])
                for si in range(T):
                    nmm = len(eterms) * T
                    i = 0
                    for stat_t, mov_t in eterms:
                        for ct in range(T):
                            if stat_t is xr:
                                st_ap = stat_t[:, ct, S * k + P * si:S * k + P * (si + 1)]
                            else:
                                st_ap = stat_t[:, ct, P * si:P * (si + 1)]
                            mv_ap = mov_t[:, ct, :]
                            nc.tensor.matmul(
                                en[si],
                                st_ap,
                                mv_ap,
                                start=(k == 0 and i == 0),
                                stop=(k == CH - 1 and i == nmm - 1),
                                skip_group_check=True,
                            )
                            i += 1

            # ---------------- softmax over j (free dim) ----------------
            for si in range(T):
                nc.vector.tensor_reduce(
                    out=negmax[:, si:si + 1], in_=en[si], axis=AX, op=maxop, negate=True
                )
                nc.scalar.activation(
                    out=attn[:, si, :],
                    in_=en[si],
                    func=Exp,
                    bias=negmax[:, si:si + 1],
                    scale=1.0,
                    accum_out=sums[:, si:si + 1],
                )
                nc.vector.reciprocal(out=rsum[:, si:si + 1], in_=sums[:, si:si + 1])
                nc.vector.tensor_scalar_mul(
                    attn[:, si, :], attn[:, si, :], rsum[:, si:si + 1]
                )

            # ---------------- phase E: y_k = gamma * (Wv Xk) attn^T + Xk --------
            def e_front(k):
                """VkT = (Wv Xk)^T via stat = Xk 128-col blocks (f32r)."""
                vkt = vktp.tile([P, T, S], f32r, name="vkt", tag="vkt")
                for ms in range(T):
                    v_ps = pss.tile([P, S], f32, tag="ps", name="v_ps")
                    for ct in range(T):
                        nc.tensor.matmul(
                            v_ps,
                            xr[:, ct, S * k + P * ms:S * k + P * (ms + 1)],
                            wvtr[:, ct, :],
                            start=(ct == 0),
                            stop=(ct == T - 1),
                        )
                    if has_bv:
                        nc.vector.tensor_tensor(
                            out=vkt[:, ms, :], in0=v_ps, in1=bvb, op=add_
                        )
                    else:
                        nc.scalar.copy(vkt[:, ms, :], v_ps)
                return vkt

            def e_back(k, vkt):
                sl = slice(S * k, S * (k + 1))
                xt2 = xtmpp.tile([P, T, S], f32, tag="xt", name="xt2")
                for t in range(T):
                    nc.sync.dma_start(
                        out=xt2[:, t, :], in_=x_d[P * t:P * (t + 1), sl]
                    )
                for os in range(T):
                    o_ps = pss.tile([P, S], f32, tag="ps", name="o_ps")
                    for jt in range(T):
                        nc.tensor.matmul(
                            o_ps,
                            vkt[:, jt, P * os:P * (os + 1)],
                            attnt[:, jt, :],
                            start=(jt == 0),
                            stop=(jt == T - 1),
                        )
                    # y = gamma * Ok + x, in place into the streamed x tile
                    nc.vector.scalar_tensor_tensor(
                        out=xt2[:, os, :],
                        in0=o_ps,
                        scalar=gammab[:, 0:1],
                        in1=xt2[:, os, :],
                        op0=mult,
                        op1=add_,
                    )
                    nc.sync.dma_start(
                        out=y_d[P * os:P * (os + 1), sl], in_=xt2[:, os, :]
                    )

            # two chunks of V-work first so PE stays busy during softmax
            pending = [e_front(0), e_front(1)]

            # attn^T via PE transposes, rounded to f32r on the copy out of PSUM
            for jt in range(T):
                for si in range(T):
                    trp = pss.tile([P, P], f32, tag="ps", name="trp")
                    nc.tensor.transpose(trp, attn[:, si, P * jt:P * (jt + 1)], ident)
                    nc.scalar.copy(attnt[:, jt, P * si:P * (si + 1)], trp)

            for k in range(CH):
                if k + 2 < CH:
                    pending.append(e_front(k + 2))
                vkt = pending.pop(0)
                e_back(k, vkt)

    nc.compile()
    return nc


_NC_CACHE = {}


def _get_nc(comp=COMP, has_bv=False):
    key = (comp, has_bv)
    if key not in _NC_CACHE:
        _NC_CACHE[key] = build(comp, has_bv)
    return _NC_CACHE[key]


def kernel(x, Wq, bq, Wk, bk, Wv, bv, gamma, comp=COMP):
    x = np.ascontiguousarray(np.asarray(x, np.float32))
    B = x.shape[0]
    assert x.shape == (B, C, 64, 64) and B == 8, x.shape
    if np.any(np.asarray(bq)) or np.any(np.asarray(bk)):
        raise NotImplementedError("nonzero q/k biases not supported")
    has_bv = bool(np.any(np.asarray(bv)))

    # host-side weight folding: M^T = Wk^T Wq in fp64 (134 MFLOP, ~0.2% of
    # the module's FLOPs) merges the q/k projections into one GEMM chain.
    mt = (np.asarray(Wk, np.float64).T @ np.asarray(Wq, np.float64)).astype(np.float32)
    wvt = np.ascontiguousarray(np.asarray(Wv, np.float32).T)
    gam = np.full((P, 1), np.float32(np.asarray(gamma).reshape(-1)[0]), np.float32)
    bvb = np.ascontiguousarray(
        np.broadcast_to(np.asarray(bv, np.float32), (P, S))
    ).astype(np.float32)

    nc = _get_nc(comp, has_bv)
    in_maps = [
        {
            "x": np.ascontiguousarray(x[b].reshape(C, N)),
            "mt": mt,
            "wvt": wvt,
            "gam": gam,
            "bvb": bvb,
        }
        for b in range(B)
    ]
    res = run_bass_kernel_spmd(nc, in_maps, core_ids=list(range(B)))
    out = np.stack([res.results[b]["y"].reshape(C, 64, 64) for b in range(B)])
    return out.astype(np.float32)
